# revision 19
# baseline (speedup 1.0000x reference)
"""Cross-attention kernel for Trainium2, SPMD over 8 NeuronCores — plan B:
pairwise k/v exchange overlapped with local-half attention.

Sharding: core i = (batch b = i//2, half h = i%2). Each core computes q^T for
its 1024 query rows and k^T/v for its OWN 1024 key rows; the pair exchanges
k/v halves with two pairwise AllGathers (k first, then v) while the core runs
q-projection and the attention of its queries against its LOCAL keys. The
remote half is then processed in a second pass and the two partial outputs /
softmax sums are combined (no max-subtraction, so partials just add).

Key order per core: kt 0-7 = own keys, kt 8-15 = peer keys. Softmax is
permutation-invariant over keys, so each core may process keys in its own
order. The peer slot of the AllGather output is selected with a
partition-id-derived dynamic DRAM offset (slot = 1 - pid%2).
"""

import json

import numpy as np

B, N, C = 4, 2048, 1024
NQ = N // 2          # query rows per core
NK = N // 2          # key rows per core (own half)
CT = C // 128        # contraction tiles
KTL = NK // 128      # local key tiles (8)
KT = N // 128        # total key tiles (16)
SCALE = 1.0 / np.sqrt(C)

_CACHE = {}


# ---------------------------------------------------------------------------
def _fix_bir(bir: dict) -> dict:
    counter = [0]
    for fn in bir.get("functions", []):
        for bb in fn.get("blocks", []):
            out = []
            for ins in bb.get("instructions", []):
                si = ins.get("sync_info") or {}
                waits = si.get("on_wait") or []
                if len(waits) > 1 and ins.get("engine") not in (None, "Unassigned"):
                    for w in waits[:-1]:
                        counter[0] += 1
                        ev = {
                            "engine": ins["engine"],
                            "ins": [],
                            "name": f"hoistwait_{counter[0]}",
                            "opcode": "EventSemaphore",
                            "outs": [],
                            "sync_info": {"on_update": [], "on_wait": [w]},
                        }
                        if "debug" in ins:
                            ev["debug"] = ins["debug"]
                        out.append(ev)
                    si["on_wait"] = [waits[-1]]
                out.append(ins)
            bb["instructions"] = out
    return bir


def _install_waitfix(nc):
    orig = nc.to_json_bytes

    def patched():
        return json.dumps(_fix_bir(json.loads(orig()))).encode()

    nc.to_json_bytes = patched


# ---------------------------------------------------------------------------
def _build():
    import concourse.bass as bass
    import concourse.tile as tile
    from concourse import mybir

    f16, f32 = mybir.dt.float16, mybir.dt.float32
    Exp = mybir.ActivationFunctionType.Exp
    Ident = mybir.ActivationFunctionType.Identity

    # All big inputs are host-packed into their exact SBUF layout [128, X]
    # so every DMA moves long contiguous rows: the DGE is packet-rate-bound
    # (~10ns/packet), so 1KB packets cap a queue at ~110GB/s while packed
    # 4KB+ packets run at full rate.
    nc = bass.Bass(enable_partition_id=True)
    ev_t = nc.dram_tensor("ev_t", [128, CT * NQ], f16, kind="ExternalInput")
    img_t = nc.dram_tensor("img_t", [2, 128, CT * 512], f16, kind="ExternalInput")
    wq = nc.dram_tensor("wq", [128, CT * C], f16, kind="ExternalInput")
    wk = nc.dram_tensor("wk", [2, 128, CT * 512], f16, kind="ExternalInput")
    wv = nc.dram_tensor("wv", [128, CT * C], f16, kind="ExternalInput")
    bqt = nc.dram_tensor("bqt", [128, CT], f32, kind="ExternalInput")
    bkt = nc.dram_tensor("bkt", [128, CT], f32, kind="ExternalInput")
    bv = nc.dram_tensor("bv", [C], f32, kind="ExternalInput")
    out = nc.dram_tensor("out", [NQ, C], f32, kind="ExternalOutput")
    RG = [[0, 1], [2, 3], [4, 5], [6, 7]]

    with tile.TileContext(nc) as tc:
        with (
            tc.tile_pool(name="ins", bufs=1) as ins_pool,
            tc.tile_pool(name="qkv", bufs=1) as qkv_pool,
            tc.tile_pool(name="expp", bufs=1) as exp_pool,
            tc.tile_pool(name="work", bufs=2) as work,
            tc.tile_pool(name="dram", bufs=1, space="DRAM") as dram_pool,
            tc.tile_pool(name="ps_a", bufs=2, space="PSUM") as ps_a,
            tc.tile_pool(name="ps_b", bufs=2, space="PSUM") as ps_b,
            tc.tile_pool(name="ps_sc", bufs=2, space="PSUM") as ps_sc,
            tc.tile_pool(name="ps_sum", bufs=2, space="PSUM") as ps_sum,
        ):
            # ---- stage A: inputs ----------------------------------------
            wq_r = wq.rearrange("p (t n) -> p t n", n=C)
            wk_r = wk.rearrange("h p (t n) -> h p t n", n=512)
            wv_r = wv.rearrange("p (t n) -> p t n", n=C)
            img_r = img_t.rearrange("h p (t n) -> h p t n", n=512)

            bq_sb = ins_pool.tile([128, CT], f32)
            bk_sb = ins_pool.tile([128, CT], f32)
            nc.scalar.dma_start(out=bk_sb[:], in_=bkt[:, :])
            nc.scalar.dma_start(out=bq_sb[:], in_=bqt[:, :])
            bv_sb = ins_pool.tile([128, C], f32)
            nc.scalar.dma_start(out=bv_sb[:], in_=bv[None, :].to_broadcast((128, C)))

            wk_sb = [
                ins_pool.tile([128, CT, 512], f16, name=f"wk{h}", tag=f"wk{h}")
                for h in range(2)
            ]
            img_sb = [
                ins_pool.tile([128, CT, 512], f16, name=f"img{ch}", tag=f"img{ch}")
                for ch in range(2)
            ]
            nc.sync.dma_start(out=wk_sb[0][:], in_=wk_r[0])
            nc.sync.dma_start(out=img_sb[0][:], in_=img_r[0])
            nc.sync.dma_start(out=wk_sb[1][:], in_=wk_r[1])
            nc.sync.dma_start(out=img_sb[1][:], in_=img_r[1])

            wv_sb = ins_pool.tile([128, CT, C], f16)
            nc.scalar.dma_start(out=wv_sb[:], in_=wv_r)
            wq_sb = ins_pool.tile([128, CT, C], f16)
            nc.scalar.dma_start(out=wq_sb[:], in_=wq_r)
            # ev shares a rotating pool slot with the exp buffers (it is
            # dead after the q-projection); flat layout matches the packed
            # host array, so this is one max-rate DMA
            ev_sb = exp_pool.tile([128, CT * NQ], f16, name="ev",
                                  tag="flat16", bufs=2)
            nc.scalar.dma_start(out=ev_sb[:], in_=ev_t[:, :])

            ones_sb = ins_pool.tile([128, 1], f16)
            nc.vector.memset(ones_sb[:], 1.0)

            # ---- PE warmup (HAM clock gate) -----------------------------
            warm_sb = ins_pool.tile([128, 512], f16)
            nc.vector.memset(warm_sb[:], 0.0)
            for w in range(32):
                ps = ps_sc.tile([128, 512], f32, tag="sc")
                nc.tensor.matmul(ps, warm_sb[:, 0:128], warm_sb[:],
                                 start=True, stop=True)

            # ---- stage B: local projections + exchange ------------------
            # k^T for own keys -> k_sb[:, :, 0:NK]; peer half lands at NK:.
            k_sb = qkv_pool.tile([128, CT, N], f16)    # k^T  [c | nk]
            v_sb = qkv_pool.tile([128, KT, C], f16)    # v    [nk | c]
            # bounce/gather buffers in SBUF-image layout [128, X]: both the
            # bounce write and the remote reload move 16KB-contiguous rows
            kv_k = dram_pool.tile([128, CT * NK], f16)
            kv_v = dram_pool.tile([128, KTL * C], f16)
            k_gath = dram_pool.tile([2, 128, CT * NK], f16)
            v_gath = dram_pool.tile([2, 128, KTL * C], f16)

            for ch in range(2):
                for co in range(CT):
                    ps = ps_a.tile([128, 512], f32, tag="a")
                    for t in range(CT):
                        nc.tensor.matmul(
                            ps,
                            wk_sb[co // 4][:, t, (co % 4) * 128:(co % 4 + 1) * 128],
                            img_sb[ch][:, t, :],
                            start=(t == 0), stop=(t == CT - 1),
                        )
                    nc.scalar.activation(
                        k_sb[:, co, ch * 512:(ch + 1) * 512], ps, Ident,
                        bias=bk_sb[:, co:co + 1],
                    )
            nc.sync.dma_start(
                out=kv_k[:, :].rearrange("p (t n) -> p t n", n=NK),
                in_=k_sb[:, :, 0:NK])
            nc.gpsimd.collective_compute(
                "AllGather", mybir.AluOpType.bypass, replica_groups=RG,
                ins=[kv_k[:, :]], outs=[k_gath[:]])

            for nr in range(KTL):
                for cch in range(C // 512):
                    ps = ps_a.tile([128, 512], f32, tag="a")
                    for t in range(CT):
                        nc.tensor.matmul(
                            ps,
                            img_sb[nr // 4][:, t, (nr % 4) * 128:(nr % 4 + 1) * 128],
                            wv_sb[:, t, cch * 512:(cch + 1) * 512],
                            start=(t == 0), stop=(t == CT - 1),
                        )
                    nc.vector.tensor_add(
                        v_sb[:, nr, cch * 512:(cch + 1) * 512], ps,
                        bv_sb[:, cch * 512:(cch + 1) * 512],
                    )
            nc.sync.dma_start(
                out=kv_v[:, :].rearrange("p (t n) -> p t n", n=C),
                in_=v_sb[:, 0:KTL, :])
            nc.gpsimd.collective_compute(
                "AllGather", mybir.AluOpType.bypass, replica_groups=RG,
                ins=[kv_v[:, :]], outs=[v_gath[:]])

            # q^T projection (overlaps the exchange)
            q_sb = qkv_pool.tile([128, CT, NQ], f16)   # q^T  [c | nq]
            for co in range(CT):
                for ch in range(NQ // 512):
                    ps = ps_a.tile([128, 512], f32, tag="a")
                    for t in range(CT):
                        nc.tensor.matmul(
                            ps,
                            wq_sb[:, t, co * 128:(co + 1) * 128],
                            ev_sb[:, t * NQ + ch * 512:t * NQ + (ch + 1) * 512],
                            start=(t == 0), stop=(t == CT - 1),
                        )
                    nc.scalar.activation(
                        q_sb[:, co, ch * 512:(ch + 1) * 512], ps, Ident,
                        bias=bq_sb[:, co:co + 1],
                    )

            # remote-half loads: dynamic peer slot (1 - pid%2), issued on
            # the Sync engine whose only later work is the output DMAs.
            peer = 1 - (nc.sync.partition_id() % 2)
            kg_r = k_gath.rearrange("s p (t n) -> s p t n", n=NK)
            vg_r = v_gath.rearrange("s p (t n) -> s p t n", n=C)
            nc.sync.dma_start(out=k_sb[:, :, NK:N], in_=kg_r[peer])
            nc.sync.dma_start(out=v_sb[:, KTL:KT, :], in_=vg_r[peer])

            # ---- stage C: attention -------------------------------------
            # pass 1: local keys (kt 0..7) while the exchange is in flight;
            # pass 2: peer keys (kt 8..15); combine partial sums at the end.
            # exp0 gets its own slot; exp1 reuses ev's slot (free after
            # q-proj) via the rotating "flat16" tag
            exp_sb = [
                exp_pool.tile([128, KT * 512], f16, name=f"exp{qc}",
                              tag="flat16", bufs=2)
                for qc in range(NQ // 512)
            ]
            o_loc = work.tile([128, NQ // 512, 4, C], f16, bufs=1)
            s_loc = work.tile([128, NQ // 512, 4], f32, bufs=1)

            def scores(qc, klo, khi):
                for kt in range(klo, khi):
                    ps = ps_sc.tile([128, 512], f32, tag="sc")
                    for t in range(CT):
                        nc.tensor.matmul(
                            ps,
                            k_sb[:, t, kt * 128:(kt + 1) * 128],
                            q_sb[:, t, qc * 512:(qc + 1) * 512],
                            start=(t == 0), stop=(t == CT - 1),
                        )
                    nc.scalar.activation(
                        exp_sb[qc][:, kt * 512:(kt + 1) * 512], ps, Exp,
                        scale=float(SCALE))

            def pv(qc, q4, klo, khi):
                # two sequential kt loops (not interleaved) so the first
                # chunk's PSUM + the row sums finish early: the DVE combine /
                # normalize for chunk 0 overlaps chunk 1's matmuls, which
                # shortens the serial tail after the very last matmul
                qlo = q4 * 128
                ps0 = ps_a.tile([128, 512], f32, tag="a")
                ps1 = ps_b.tile([128, 512], f32, tag="b")
                pss = ps_sum.tile([128, 1], f32, tag="s")
                for kt in range(klo, khi):
                    st, sp = (kt == klo), (kt == khi - 1)
                    p_blk = exp_sb[qc][:, kt * 512 + qlo:kt * 512 + qlo + 128]
                    nc.tensor.matmul(ps0, p_blk, v_sb[:, kt, 0:512],
                                     start=st, stop=sp)
                    nc.tensor.matmul(pss, p_blk, ones_sb[:],
                                     start=st, stop=sp)
                for kt in range(klo, khi):
                    st, sp = (kt == klo), (kt == khi - 1)
                    p_blk = exp_sb[qc][:, kt * 512 + qlo:kt * 512 + qlo + 128]
                    nc.tensor.matmul(ps1, p_blk, v_sb[:, kt, 512:1024],
                                     start=st, stop=sp)
                return ps0, ps1, pss

            # pass 1 (local)
            for qc in range(NQ // 512):
                scores(qc, 0, KTL)
            for qc in range(NQ // 512):
                for q4 in range(4):
                    ps0, ps1, pss = pv(qc, q4, 0, KTL)
                    nc.vector.tensor_copy(s_loc[:, qc, q4:q4 + 1], pss)
                    nc.vector.tensor_copy(o_loc[:, qc, q4, 0:512], ps0)
                    nc.vector.tensor_copy(o_loc[:, qc, q4, 512:1024], ps1)

            # pass 2 (remote) + combine
            for qc in range(NQ // 512):
                scores(qc, KTL, KT)
            for qc in range(NQ // 512):
                for q4 in range(4):
                    qlo = q4 * 128
                    ps0, ps1, pss = pv(qc, q4, KTL, KT)
                    stot = work.tile([128, 1], f32, tag="stot")
                    nc.vector.tensor_add(stot[:], pss, s_loc[:, qc, q4:q4 + 1])
                    recip = work.tile([128, 1], f32, tag="recip")
                    nc.vector.reciprocal(recip[:], stot[:])
                    for cch, psv in ((0, ps0), (1, ps1)):
                        o_sb = work.tile([128, 512], f32, tag=f"o{cch}")
                        nc.vector.tensor_add(
                            o_sb[:], psv,
                            o_loc[:, qc, q4, cch * 512:(cch + 1) * 512])
                        nc.vector.tensor_scalar_mul(o_sb[:], o_sb[:], recip[:])
                        nc.sync.dma_start(
                            out=out[qc * 512 + qlo:qc * 512 + qlo + 128,
                                    cch * 512:(cch + 1) * 512],
                            in_=o_sb[:],
                        )
    _install_waitfix(nc)
    return nc


def _get_nc():
    if "nc" not in _CACHE:
        _CACHE["nc"] = _build()
    return _CACHE["nc"]


def run(inputs, trace=False, trace_cores=None):
    from concourse.bass_utils import run_bass_kernel_spmd

    event_f = np.asarray(inputs["event_f"], dtype=np.float32)
    img_f = np.asarray(inputs["img_f"], dtype=np.float32)
    Wq = np.asarray(inputs["Wq"], dtype=np.float32).astype(np.float16)
    Wk = np.asarray(inputs["Wk"], dtype=np.float32).astype(np.float16)
    Wv = np.asarray(inputs["Wv"], dtype=np.float32).astype(np.float16)
    bq = np.asarray(inputs["bq"], dtype=np.float32)
    bk = np.asarray(inputs["bk"], dtype=np.float32)
    bv = np.asarray(inputs["bv"], dtype=np.float32)
    bqt = np.ascontiguousarray(bq.reshape(CT, 128).T)
    bkt = np.ascontiguousarray(bk.reshape(CT, 128).T)

    def pack(a_t):  # [C, X] feature-major -> SBUF image [128, CT*X]
        x = a_t.shape[1]
        return np.ascontiguousarray(
            a_t.reshape(CT, 128, x).transpose(1, 0, 2).reshape(128, CT * x))

    wq_p = pack(Wq)           # lhsT layout == natural [C, C] feature-major
    wv_p = pack(Wv)
    wk_p = np.ascontiguousarray(
        Wk.reshape(CT, 128, 2, 512).transpose(2, 1, 0, 3).reshape(2, 128, CT * 512))

    in_maps = []
    for core in range(8):
        b, h = core // 2, core % 2
        ev_tc = pack(event_f[b, h * NQ:(h + 1) * NQ, :].T.astype(np.float16))
        img_h = img_f[b].T[:, h * NK:(h + 1) * NK].astype(np.float16)
        img_tc = np.ascontiguousarray(
            img_h.reshape(CT, 128, 2, 512).transpose(2, 1, 0, 3)
            .reshape(2, 128, CT * 512))
        in_maps.append({
            "ev_t": ev_tc, "img_t": img_tc,
            "wq": wq_p, "wk": wk_p, "wv": wv_p,
            "bqt": bqt, "bkt": bkt, "bv": bv,
        })

    nc = _get_nc()
    res = run_bass_kernel_spmd(
        nc, in_maps, list(range(8)), trace=trace,
        **({"trace_cores": trace_cores} if trace_cores else {}),
    )
    full = np.empty((B, N, C), dtype=np.float32)
    for core in range(8):
        b, h = core // 2, core % 2
        full[b, h * NQ:(h + 1) * NQ, :] = res.results[core]["out"]
    return full, res


def kernel(**inputs) -> np.ndarray:
    full, _ = run(inputs, trace=False)
    return full
